# revision 1
# baseline (speedup 1.0000x reference)
import time
import numpy as np
from contextlib import ExitStack

import jax
from jax.experimental.shard_map import shard_map
from jax.sharding import Mesh, PartitionSpec, NamedSharding

import concourse.bass as bass
import concourse.bacc as bacc
import concourse.tile as tile
import concourse.mybir as mybir
from concourse._compat import with_exitstack
from concourse.bass2jax import _bass_exec_p, install_neuronx_cc_hook, partition_id_tensor

F32 = mybir.dt.float32

NCORES = 8
BATCH = 32768
B_LOCAL = BATCH // NCORES
D = 16
CH1 = 64
CH2 = 32
RANK = 64
BLK = 128


def _emit(ctx: ExitStack, tc: tile.TileContext, outs, ins, b_local: int):
    nc = tc.nc
    (out_d,) = outs
    (x1_d, x2_d, w_d, a_d, b_d, ct_d) = ins
    nblk = b_local // BLK

    const = ctx.enter_context(tc.tile_pool(name="const", bufs=1))
    A_sb = const.tile([128, CH1], F32)
    B_sb = const.tile([128, RANK], F32)
    CT_sb = const.tile([128, 32], F32)
    for rp in range(4):
        nc.sync.dma_start(A_sb[32 * rp:32 * rp + 16, :], a_d[:, :])
        nc.sync.dma_start(B_sb[32 * rp:32 * rp + 16, :], b_d[:, :])
    for rp2 in (0, 64):
        for cj in (0, 16):
            nc.sync.dma_start(CT_sb[rp2:rp2 + 64, cj:cj + 16], ct_d[:, :])

    x1_pool = ctx.enter_context(tc.tile_pool(name="x1", bufs=2))
    x2_pool = ctx.enter_context(tc.tile_pool(name="x2", bufs=2))
    w_pool = ctx.enter_context(tc.tile_pool(name="w", bufs=2))
    x2T_pool = ctx.enter_context(tc.tile_pool(name="x2T", bufs=2))
    wT_pool = ctx.enter_context(tc.tile_pool(name="wT", bufs=2))
    qsb_pool = ctx.enter_context(tc.tile_pool(name="qsb", bufs=2))
    m_pool = ctx.enter_context(tc.tile_pool(name="m", bufs=4))
    osb_pool = ctx.enter_context(tc.tile_pool(name="osb", bufs=3))
    pq = ctx.enter_context(tc.tile_pool(name="pq", bufs=1, space="PSUM"))
    pt = ctx.enter_context(tc.tile_pool(name="pt", bufs=6, space="PSUM"))
    po = ctx.enter_context(tc.tile_pool(name="po", bufs=1, space="PSUM"))
    t3sb_pool = ctx.enter_context(tc.tile_pool(name="t3sb", bufs=2))

    for blk in range(nblk):
        b0 = blk * BLK
        x1_t = x1_pool.tile([128, 2048], F32)
        for rp in range(4):
            src = x1_d[b0 + 32 * rp:b0 + 32 * rp + 32, :, :].rearrange("b d o -> d b o")
            dst = x1_t[32 * rp:32 * rp + 16, :].rearrange("p (b o) -> p b o", o=64)
            nc.sync.dma_start(dst, src)
        x2_t = x2_pool.tile([128, 512], F32)
        for g in range(4):
            for pr in range(2):
                src = x2_d[b0 + 32 * g + pr:b0 + 32 * (g + 1):2, :, :].rearrange("c j v -> j c v")
                dst = x2_t[32 * g + 16 * pr:32 * g + 16 * pr + 16, :].rearrange("p (c v) -> p c v", v=32)
                nc.sync.dma_start(dst, src)
        w_t = w_pool.tile([128, 2048], F32)
        for g in range(4):
            for h in range(2):
                src = w_d[b0 + 32 * g:b0 + 32 * g + 32, 32 * h:32 * h + 32, :].rearrange("s p v -> p s v")
                dst = w_t[32 * g:32 * g + 32, :].rearrange("p (s hv) -> p s hv", hv=64)[:, :, 32 * h:32 * h + 32]
                nc.sync.dma_start(dst, src)

        x2T_t = x2T_pool.tile([128, 544], F32)
        nc.vector.transpose(x2T_t[:, 0:512], x2_t[:])
        nc.vector.memset(x2T_t[:, 512:544], 0.0)
        wT_t = wT_pool.tile([128, 2048], F32)
        nc.vector.transpose(wT_t[:], w_t[:])

        q_sb = qsb_pool.tile([128, 2048], F32)
        for s in range(4):
            bs = b0 + 32 * s
            q_ps = pq.tile([128, 512], F32)
            for k in range(8):
                for j in range(4):
                    bb = 8 * j + k
                    x = 32 * (bb // 2) + 16 * (bb % 2)
                    nc.tensor.matmul(
                        q_ps[32 * j:32 * j + 32, 64 * k:64 * k + 64],
                        x2T_t[32 * s:32 * s + 32, x:x + 32],
                        wT_t[32 * s:32 * s + 32, 64 * bb:64 * bb + 64],
                        tile_position=(32 * s, 32 * j),
                    )
            nc.scalar.copy(q_sb[:, 512 * s:512 * (s + 1)], q_ps[:])

            t1a = pt.tile([128, 512], F32, tag="t")
            t1b = pt.tile([128, 512], F32, tag="t")
            t3a = pt.tile([128, 512], F32, tag="t")
            t3b = pt.tile([128, 512], F32, tag="t")
            t1x = [t1a, t1a, t1b, t1b]
            t3x = [t3a, t3a, t3b, t3b]
            for j in range(4):
                cp = 64 * (j % 2)
                nc.tensor.matmul(
                    t1x[j][cp:cp + 64, :],
                    A_sb[32 * s:32 * s + 16, :],
                    x1_t[32 * s:32 * s + 16, 512 * j:512 * (j + 1)],
                    tile_position=(32 * s, cp),
                )
                nc.tensor.matmul(
                    t3x[j][cp:cp + 64, :],
                    B_sb[32 * j:32 * j + 16, :],
                    q_sb[32 * j:32 * j + 16, 512 * s:512 * (s + 1)],
                    tile_position=(32 * j, cp),
                )

            t3_sb = t3sb_pool.tile([128, 1024], F32)
            nc.scalar.copy(t3_sb[:, 0:512], t3a[:])
            nc.scalar.copy(t3_sb[:, 512:1024], t3b[:])
            m_t = m_pool.tile([128, 1024], F32)
            nc.vector.tensor_mul(m_t[:, 0:512], t1a[:], t3_sb[:, 0:512])
            nc.vector.tensor_mul(m_t[:, 512:1024], t1b[:], t3_sb[:, 512:1024])

            o_ps = po.tile([128, 512], F32)
            for j in range(4):
                rp2 = 64 * (j % 2)
                nc.tensor.matmul(
                    o_ps[32 * j:32 * j + 32, :],
                    CT_sb[rp2:rp2 + 64, :],
                    m_t[rp2:rp2 + 64, 512 * (j // 2):512 * (j // 2) + 512],
                    tile_position=(rp2, 32 * j),
                )
            o_sb = osb_pool.tile([128, 512], F32)
            nc.scalar.copy(o_sb[:], o_ps[:])
            for j in range(4):
                dst = out_d[bs + 8 * j:bs + 8 * j + 8, :, :].rearrange("k c o -> c k o")
                src = o_sb[32 * j:32 * j + 16, :].rearrange("p (k o) -> p k o", o=64)
                nc.sync.dma_start(dst, src)


@with_exitstack
def _cp_kernel(ctx, tc, outs, ins, b_local):
    _emit(ctx, tc, outs, ins, b_local)


def build_nc(b_local: int = B_LOCAL):
    nc = bacc.Bacc("TRN2", target_bir_lowering=False, debug=False)
    x1_d = nc.dram_tensor("x1", [b_local, D, CH1], F32, kind="ExternalInput").ap()
    x2_d = nc.dram_tensor("x2", [b_local, D, CH2], F32, kind="ExternalInput").ap()
    w_d = nc.dram_tensor("w", [b_local, CH1, CH2], F32, kind="ExternalInput").ap()
    a_d = nc.dram_tensor("a", [D, RANK], F32, kind="ExternalInput").ap()
    b_d = nc.dram_tensor("b", [D, RANK], F32, kind="ExternalInput").ap()
    ct_d = nc.dram_tensor("ct", [RANK, D], F32, kind="ExternalInput").ap()
    out_d = nc.dram_tensor("out", [b_local, D, CH1], F32, kind="ExternalOutput").ap()
    with tile.TileContext(nc, trace_sim=False) as tc:
        _cp_kernel(tc, [out_d], [x1_d, x2_d, w_d, a_d, b_d, ct_d], b_local)
    nc.compile()
    return nc


class _SpmdRunner:

    def __init__(self, nc, n_cores=NCORES):
        install_neuronx_cc_hook()
        self.nc = nc
        self.n_cores = n_cores
        pid_name = nc.partition_id_tensor.name if nc.partition_id_tensor else None

        in_names, out_names, out_avals, zero_outs = [], [], [], []
        for alloc in nc.m.functions[0].allocations:
            if not isinstance(alloc, mybir.MemoryLocationSet):
                continue
            name = alloc.memorylocations[0].name
            if alloc.kind == "ExternalInput":
                if name != pid_name:
                    in_names.append(name)
            elif alloc.kind == "ExternalOutput":
                out_names.append(name)
                shape = tuple(alloc.tensor_shape)
                dtype = mybir.dt.np(alloc.dtype)
                out_avals.append(jax.core.ShapedArray(shape, dtype))
                zero_outs.append(np.zeros(shape, dtype))
        self.in_names, self.out_names = in_names, out_names
        self.out_avals, self.zero_outs = out_avals, zero_outs
        n_params = len(in_names)
        all_names = tuple(in_names + out_names + ([pid_name] if pid_name else []))

        def _body(*args):
            operands = list(args)
            if pid_name is not None:
                operands.append(partition_id_tensor())
            outs = _bass_exec_p.bind(
                *operands,
                out_avals=tuple(out_avals),
                in_names=all_names,
                out_names=tuple(out_names),
                lowering_input_output_aliases=(),
                sim_require_finite=True,
                sim_require_nnan=True,
                nc=nc,
            )
            return tuple(outs)

        devices = jax.devices()[:n_cores]
        self.mesh = Mesh(np.asarray(devices), ("core",))
        self.sharding = NamedSharding(self.mesh, PartitionSpec("core"))
        n_out = len(out_names)
        donate = tuple(range(n_params, n_params + n_out))
        self.jitted = jax.jit(
            shard_map(_body, mesh=self.mesh,
                      in_specs=(PartitionSpec("core"),) * (n_params + n_out),
                      out_specs=(PartitionSpec("core"),) * n_out,
                      check_rep=False),
            donate_argnums=donate, keep_unused=True,
        )

    def stage_inputs(self, in_maps):
        per_core = [[np.asarray(m[name]) for name in self.in_names] for m in in_maps]
        concat = [np.concatenate([per_core[c][i] for c in range(self.n_cores)], axis=0)
                  for i in range(len(self.in_names))]
        return [jax.device_put(a, self.sharding) for a in concat]

    def stage_zeros(self):
        zs = [np.zeros((self.n_cores * z.shape[0], *z.shape[1:]), z.dtype)
              for z in self.zero_outs]
        return [jax.device_put(z, self.sharding) for z in zs]

    def run(self, dev_inputs, dev_zeros=None):
        if dev_zeros is None:
            dev_zeros = self.stage_zeros()
        outs = self.jitted(*dev_inputs, *dev_zeros)
        jax.block_until_ready(outs)
        return outs

    def unshard_out(self, outs):
        i = self.out_names.index("out")
        a = np.asarray(outs[i])
        return a


_RUNNER = None


def _get_runner():
    global _RUNNER
    if _RUNNER is None:
        nc = build_nc(B_LOCAL)
        _RUNNER = _SpmdRunner(nc, NCORES)
    return _RUNNER


def kernel(x1, x2, w, A, B, C):
    runner = _get_runner()
    x1 = np.ascontiguousarray(np.asarray(x1, dtype=np.float32))
    x2 = np.ascontiguousarray(np.asarray(x2, dtype=np.float32))
    w = np.ascontiguousarray(np.asarray(w, dtype=np.float32))
    A = np.ascontiguousarray(np.asarray(A, dtype=np.float32))
    B = np.ascontiguousarray(np.asarray(B, dtype=np.float32))
    CT = np.ascontiguousarray(np.asarray(C, dtype=np.float32).T)

    bl = x1.shape[0] // NCORES
    in_maps = []
    for c in range(NCORES):
        sl = slice(c * bl, (c + 1) * bl)
        in_maps.append({"x1": x1[sl], "x2": x2[sl], "w": w[sl],
                        "a": A, "b": B, "ct": CT})
    dev_in = runner.stage_inputs(in_maps)
    outs = runner.run(dev_in)
    return runner.unshard_out(outs)

